# revision 13
# baseline (speedup 1.0000x reference)
"""Trainium2 Bass kernel for nn_ELECT_Mnist (GIN message passing + ELECT loss).

Strategy (8 NeuronCores, data-parallel over graphs, 32 graphs/core):
 - Dense per-graph adjacency built on device via one-hot matmuls (edges are
   graph-local), so GIN aggregation becomes one [128,128]@[128,512] matmul
   per graph instead of irregular gather/scatter.
 - GIN MLPs run feature-major on the tensor engine; BatchNorm batch stats
   are global over all 32768 nodes -> per-core partials + AllReduce.
 - GraphSizeNorm (x / sqrt(128)) is folded into the following BatchNorm
   exactly (uniform graph size), by scaling the BN eps by 128.
 - Spectral part: per-graph 128x128 Laplacian eigenvalues via batched
   Householder tridiagonalization (graph-per-partition layout) + multi-
   section Sturm bisection, fully on device.
Self-contained: hardcodes shapes from the problem spec.
"""

import numpy as np

G, NP, E_HALF = 256, 128, 1024
D = 512
NCORES = 8
GC = G // NCORES              # 32 graphs per core
NODES_C = GC * NP             # 4096 nodes per core
HALF_N = NODES_C // 2         # 2048
EC = GC * E_HALF              # 32768 (undirected-half) edges per core
N = G * NP
BN_EPS = 1e-5
LRELU_SLOPE = 0.01
HH_STEPS = NP - 2
BIS_S = 16
BIS_PASSES = 5
NPROB = 3 * GC                # 96 bisection problems per core

_cache = {}


def _build_program(consts_in, uniform_counts):
    import concourse.bass as bass
    import concourse.mybir as mybir
    import concourse.tile as tile
    from concourse.masks import make_identity
    from waitsplit import split_waits

    F32 = mybir.dt.float32
    U32 = mybir.dt.uint32
    AL = mybir.AluOpType
    AF = mybir.ActivationFunctionType
    AX = mybir.AxisListType

    assert uniform_counts, "non-uniform graph sizes not supported by this kernel"
    e1, e2, e3, lin2b = consts_in

    nc = bass.Bass(num_devices=NCORES)

    t_lr = nc.dram_tensor("lr", [EC], F32, kind="ExternalInput")
    t_lc = nc.dram_tensor("lc", [EC], F32, kind="ExternalInput")
    t_ew = nc.dram_tensor("ew", [EC], F32, kind="ExternalInput")
    t_x = nc.dram_tensor("x0", [NODES_C], F32, kind="ExternalInput")
    t_Wm = nc.dram_tensor("Wm", [5, D, D], F32, kind="ExternalInput")
    t_l1w = nc.dram_tensor("lin1w", [D, 64], F32, kind="ExternalInput")
    t_vecs = nc.dram_tensor("vecs", [21, D], F32, kind="ExternalInput")
    o_p = nc.dram_tensor("p_out", [NODES_C], F32, kind="ExternalOutput")
    o_loss = nc.dram_tensor("loss", [GC], F32, kind="ExternalOutput")
    o_fr = nc.dram_tensor("f_relax", [GC], F32, kind="ExternalOutput")
    o_gr = nc.dram_tensor("g_relax", [GC], F32, kind="ExternalOutput")
    o_ev = nc.dram_tensor("ev_dbg", [NPROB], F32, kind="ExternalOutput")
    o_ab = nc.dram_tensor("ab_dbg", [2, 2 * GC, NP], F32, kind="ExternalOutput")
    o_lap = nc.dram_tensor("lap_dbg", [2 * GC, NP * NP], F32, kind="ExternalOutput")

    V_W1, V_B1, V_B2 = 0, 1, 2
    V_C0B1, V_C0B2, V_C1B1, V_C1B2 = 3, 4, 5, 6
    V_GBN = {"gin1": (7, 8), "bn1": (9, 10), "gin2": (11, 12),
             "bns0": (13, 14), "gin3": (15, 16), "bns1": (17, 18)}
    V_L1B, V_L2W = 19, 20

    with tile.TileContext(nc) as tc:
        import contextlib
        ctx = contextlib.ExitStack()
        with ctx:
            consts = ctx.enter_context(tc.tile_pool(name="consts", bufs=1))
            dram = ctx.enter_context(tc.tile_pool(name="dram", bufs=1, space="DRAM"))
            big = ctx.enter_context(tc.tile_pool(name="big", bufs=1))
            sp = ctx.enter_context(tc.tile_pool(name="sp", bufs=2))
            mid = ctx.enter_context(tc.tile_pool(name="mid", bufs=1))
            rows = ctx.enter_context(tc.tile_pool(name="rows", bufs=1))
            spA = ctx.enter_context(tc.tile_pool(name="spA", bufs=2))
            pp = ctx.enter_context(tc.tile_pool(name="pp", bufs=2, space="PSUM"))
            ppB = ctx.enter_context(tc.tile_pool(name="ppB", bufs=2, space="PSUM"))
            ppF = ctx.enter_context(tc.tile_pool(name="ppF", bufs=1, space="PSUM"))

            ident = consts.tile([128, 128], F32)
            make_identity(nc, ident[:])
            iota_t = consts.tile([128, 128], F32)
            nc.gpsimd.iota(iota_t[:], pattern=[[1, 128]], base=0,
                           channel_multiplier=0, allow_small_or_imprecise_dtypes=True)
            ones_c = consts.tile([128, 1], F32)
            nc.vector.memset(ones_c[:], 1.0)
            ones_r = consts.tile([1, 128], F32)
            nc.vector.memset(ones_r[:], 1.0)
            vc = consts.tile([128, 21, 4], F32)
            nc.sync.dma_start(out=vc[:], in_=t_vecs[:].rearrange("v (m p) -> p v m", p=128))

            Adr = dram.tile([GC, NP, NP], F32)
            Wdr = dram.tile([GC, NP, NP], F32)
            lapdr = dram.tile([2 * GC, NP * NP], F32)
            z0dr = dram.tile([NODES_C], F32)
            hladr = dram.tile([NODES_C], F32)
            lamdr = dram.tile([NPROB], F32)
            abdr = dram.tile([2, 2 * GC, NP], F32)
            bn_in = dram.tile([128, 8], F32)
            bn_outs = [dram.tile([128, 8], F32, addr_space="Shared",
                                 name=f"bn_out{i}", tag=f"bn_out{i}")
                       for i in range(6)]
            bn_ctr = [0]

            # ---------------- Phase A: adjacency build ----------------
            lr_sb = consts.tile([128, GC, 8], F32)
            lc_sb = consts.tile([128, GC, 8], F32)
            ew_sb = consts.tile([128, GC, 8], F32)
            nc.sync.dma_start(out=lr_sb[:], in_=t_lr[:].rearrange("(g t p) -> p g t", p=128, t=8))
            nc.sync.dma_start(out=lc_sb[:], in_=t_lc[:].rearrange("(g t p) -> p g t", p=128, t=8))
            nc.sync.dma_start(out=ew_sb[:], in_=t_ew[:].rearrange("(g t p) -> p g t", p=128, t=8))

            for g in range(GC):
                Cp = pp.tile([128, 128], F32, tag="psA")
                Cwp = pp.tile([128, 128], F32, tag="psB")
                for t in range(8):
                    ohr = spA.tile([128, 128], F32, tag="ohr")
                    ohc = spA.tile([128, 128], F32, tag="ohc")
                    ohrw = spA.tile([128, 128], F32, tag="ohrw")
                    nc.vector.tensor_scalar(out=ohr[:], in0=iota_t[:],
                                            scalar1=lr_sb[:, g, t:t + 1], scalar2=None,
                                            op0=AL.is_equal)
                    nc.vector.tensor_scalar(out=ohc[:], in0=iota_t[:],
                                            scalar1=lc_sb[:, g, t:t + 1], scalar2=None,
                                            op0=AL.is_equal)
                    nc.vector.tensor_scalar(out=ohrw[:], in0=ohr[:],
                                            scalar1=ew_sb[:, g, t:t + 1], scalar2=None,
                                            op0=AL.mult)
                    nc.tensor.matmul(out=Cp[:], lhsT=ohr[:], rhs=ohc[:],
                                     start=(t == 0), stop=(t == 7))
                    nc.tensor.matmul(out=Cwp[:], lhsT=ohrw[:], rhs=ohc[:],
                                     start=(t == 0), stop=(t == 7))
                for (P_, dst) in ((Cp, Adr), (Cwp, Wdr)):
                    Cs = sp.tile([128, 128], F32, tag="Cs")
                    nc.vector.tensor_copy(out=Cs[:], in_=P_[:])
                    Tp = pp.tile([128, 128], F32, tag="psA")
                    nc.tensor.transpose(out=Tp[:], in_=Cs[:], identity=ident[:])
                    As = sp.tile([128, 128], F32, tag="As")
                    nc.vector.tensor_tensor(out=As[:], in0=Cs[:], in1=Tp[:], op=AL.add)
                    nc.sync.dma_start(out=dst[g], in_=As[:])

            # ---------------- persistent GNN tiles ----------------
            h_fm = big.tile([128, 4, NODES_C], F32, tag="h_fm")   # 8MB
            zdr = dram.tile([128, 4, NODES_C], F32)               # z mirror in DRAM

            def bn_stats_allreduce(load_chunk, eps_eff, key):
                """load_chunk(q) -> [128,4,512] AP for node-chunk q. Returns
                per-chunk (scl[m], shf[m]) tiles after global AllReduce."""
                gi, bi = V_GBN[key]
                stats = sp.tile([128, 8], F32, tag="bnstats")
                nc.vector.memset(stats[:], 0.0)
                for q in range(8):
                    zc = load_chunk(q)
                    for m in range(4):
                        part = sp.tile([128, 1], F32, tag="bnpart")
                        nc.vector.tensor_reduce(part[:], zc[:, m, :], AX.X, AL.add)
                        nc.vector.tensor_tensor(out=stats[:, 2 * m:2 * m + 1],
                                                in0=stats[:, 2 * m:2 * m + 1],
                                                in1=part[:], op=AL.add)
                        part2 = sp.tile([128, 1], F32, tag="bnpart2")
                        sqs = mid.tile([128, 512], F32, tag="sqs")
                        nc.scalar.activation(out=sqs[:], in_=zc[:, m, :], func=AF.Square)
                        nc.vector.tensor_reduce(part2[:], sqs[:], AX.X, AL.add)
                        nc.vector.tensor_tensor(out=stats[:, 2 * m + 1:2 * m + 2],
                                                in0=stats[:, 2 * m + 1:2 * m + 2],
                                                in1=part2[:], op=AL.add)
                nc.sync.dma_start(out=bn_in[:], in_=stats[:])
                bno = bn_outs[bn_ctr[0]]; bn_ctr[0] += 1
                nc.gpsimd.collective_compute(
                    "AllReduce", AL.add, replica_groups=[list(range(NCORES))],
                    ins=[bn_in[:].opt()], outs=[bno[:].opt()])
                gstats = sp.tile([128, 8], F32, tag="bngst")
                nc.sync.dma_start(out=gstats[:], in_=bno[:])
                epst = sp.tile([128, 1], F32, tag="bneps")
                nc.vector.memset(epst[:], float(eps_eff))
                scls, shfs = [], []
                for m in range(4):
                    mu = sp.tile([128, 1], F32, tag=f"bnmu{m}")
                    nc.vector.tensor_scalar(out=mu[:], in0=gstats[:, 2 * m:2 * m + 1],
                                            scalar1=float(1.0 / N), scalar2=None, op0=AL.mult)
                    var = sp.tile([128, 1], F32, tag=f"bnvar{m}")
                    nc.vector.tensor_scalar(out=var[:], in0=gstats[:, 2 * m + 1:2 * m + 2],
                                            scalar1=float(1.0 / N), scalar2=None, op0=AL.mult)
                    mu2 = sp.tile([128, 1], F32, tag=f"bnmu2{m}")
                    nc.vector.tensor_tensor(out=mu2[:], in0=mu[:], in1=mu[:], op=AL.mult)
                    nc.vector.tensor_tensor(out=var[:], in0=var[:], in1=mu2[:], op=AL.subtract)
                    rstd = sp.tile([128, 1], F32, tag=f"bnrstd{m}")
                    nc.scalar.activation(out=rstd[:], in_=var[:], func=AF.Sqrt,
                                         bias=epst[:], scale=1.0)
                    nc.vector.reciprocal(out=rstd[:], in_=rstd[:])
                    scl = sp.tile([128, 1], F32, tag=f"bnscl{m}")
                    nc.vector.tensor_tensor(out=scl[:], in0=rstd[:], in1=vc[:, gi, m:m + 1],
                                            op=AL.mult)
                    shf = sp.tile([128, 1], F32, tag=f"bnshf{m}")
                    nc.vector.tensor_tensor(out=shf[:], in0=mu[:], in1=scl[:], op=AL.mult)
                    nc.vector.tensor_tensor(out=shf[:], in0=vc[:, bi, m:m + 1], in1=shf[:],
                                            op=AL.subtract)
                    scls.append(scl); shfs.append(shf)
                return scls, shfs

            def mlp_two_layers(w1_idx, b1_row, w2_idx, b2_row, src_is_z0):
                """z2 = relu(l2(relu(l1(z)))) chunk-wise; z read/written via zdr.
                src_is_z0: first conv reads z0row (din=1) instead of zdr."""
                for q in range(8):
                    n0 = q * 512
                    z1q = mid.tile([128, 4, 512], F32, tag="z1q")
                    if src_is_z0:
                        for m in range(4):
                            ps = pp.tile([128, 512], F32, tag="psA")
                            nc.tensor.matmul(out=ps[:], lhsT=w1row[:, m * 128:(m + 1) * 128],
                                             rhs=z0row[:, n0:n0 + 512],
                                             start=True, stop=True)
                            nc.scalar.activation(out=z1q[:, m, :], in_=ps[:], func=AF.Relu,
                                                 bias=vc[:, b1_row, m:m + 1], scale=1.0)
                    else:
                        zq = mid.tile([128, 4, 512], F32, tag="zio")
                        nc.sync.dma_start(out=zq[:], in_=zdr[:, :, n0:n0 + 512])
                        for m2 in range(4):
                            psq = ppF.tile([128, 512], F32, tag="psF")
                            for k in range(4):
                                lw = spA.tile([128, 128], F32, tag="lw")
                                nc.sync.dma_start(
                                    out=lw[:],
                                    in_=t_Wm[w1_idx, k * 128:(k + 1) * 128,
                                             m2 * 128:(m2 + 1) * 128])
                                nc.tensor.matmul(out=psq[:], lhsT=lw[:],
                                                 rhs=zq[:, k, :],
                                                 start=(k == 0), stop=(k == 3))
                            nc.scalar.activation(out=z1q[:, m2, :], in_=psq[:], func=AF.Relu,
                                                 bias=vc[:, b1_row, m2:m2 + 1], scale=1.0)
                    z2q = mid.tile([128, 4, 512], F32, tag="z2q")
                    for m2 in range(4):
                        psq = ppF.tile([128, 512], F32, tag="psF")
                        for k in range(4):
                            lw = spA.tile([128, 128], F32, tag="lw")
                            nc.sync.dma_start(
                                out=lw[:],
                                in_=t_Wm[w2_idx, k * 128:(k + 1) * 128,
                                         m2 * 128:(m2 + 1) * 128])
                            nc.tensor.matmul(out=psq[:], lhsT=lw[:],
                                             rhs=z1q[:, k, :],
                                             start=(k == 0), stop=(k == 3))
                        nc.scalar.activation(out=z2q[:, m2, :], in_=psq[:], func=AF.Relu,
                                             bias=vc[:, b2_row, m2:m2 + 1], scale=1.0)
                    nc.sync.dma_start(out=zdr[:, :, n0:n0 + 512], in_=z2q[:])

            def zdr_chunk(q):
                zc = mid.tile([128, 4, 512], F32, tag="zio")
                nc.sync.dma_start(out=zc[:], in_=zdr[:, :, q * 512:(q + 1) * 512])
                return zc

            def hfm_chunk(q):
                return h_fm[:, :, q * 512:(q + 1) * 512]

            # ---------------- conv1 ----------------
            x_sb = sp.tile([128, GC], F32, tag="xsb")
            nc.sync.dma_start(out=x_sb[:], in_=t_x[:].rearrange("(g i) -> i g", i=128))
            agg0 = pp.tile([128, GC], F32, tag="psA")
            for g in range(GC):
                Ag = spA.tile([128, 128], F32, tag="Ast")
                nc.sync.dma_start(out=Ag[:], in_=Adr[g])
                nc.tensor.matmul(out=agg0[:, g:g + 1], lhsT=Ag[:], rhs=x_sb[:, g:g + 1],
                                 start=True, stop=True)
            z0 = sp.tile([128, GC], F32, tag="z0")
            nc.vector.tensor_scalar(out=z0[:], in0=x_sb[:], scalar1=float(e1),
                                    scalar2=None, op0=AL.mult)
            nc.vector.tensor_tensor(out=z0[:], in0=z0[:], in1=agg0[:], op=AL.add)
            nc.sync.dma_start(out=z0dr[:].rearrange("(g i) -> i g", i=128), in_=z0[:])
            z0row = rows.tile([1, NODES_C], F32, tag="rowA")
            nc.sync.dma_start(out=z0row[:], in_=z0dr[:].rearrange("(o n) -> o n", o=1))
            w1row = sp.tile([1, D], F32, tag="w1row")
            nc.sync.dma_start(out=w1row[:], in_=t_vecs[V_W1].rearrange("(o n) -> o n", o=1))

            mlp_two_layers(None, V_B1, 0, V_B2, src_is_z0=True)
            scls, shfs = bn_stats_allreduce(zdr_chunk, BN_EPS, "gin1")
            for q in range(8):
                zc = zdr_chunk(q)
                for m in range(4):
                    nc.vector.tensor_scalar(out=zc[:, m, :], in0=zc[:, m, :],
                                            scalar1=scls[m][:], scalar2=shfs[m][:],
                                            op0=AL.mult, op1=AL.add)
                    nc.scalar.activation(out=h_fm[:, m, q * 512:(q + 1) * 512],
                                         in_=zc[:, m, :], func=AF.Lrelu, alpha=LRELU_SLOPE)
            scls, shfs = bn_stats_allreduce(hfm_chunk, BN_EPS * NP, "bn1")
            for m in range(4):
                nc.vector.tensor_scalar(out=h_fm[:, m, :], in0=h_fm[:, m, :],
                                        scalar1=scls[m][:], scalar2=shfs[m][:],
                                        op0=AL.mult, op1=AL.add)

            # ---------------- conv loops ----------------
            for (wm1, wm2, b1r, b2r, ginkey, bnskey, ee) in (
                    (1, 2, V_C0B1, V_C0B2, "gin2", "bns0", e2),
                    (3, 4, V_C1B1, V_C1B2, "gin3", "bns1", e3)):
                # z = e*h + A@h (via per-graph transposes), written to zdr
                for g in range(GC):
                    tp = pp.tile([128, 4, 128], F32, tag="psA")
                    for m in range(4):
                        nc.tensor.transpose(out=tp[:, m, :],
                                            in_=h_fm[:, m, g * 128:(g + 1) * 128],
                                            identity=ident[:])
                    hng = sp.tile([128, 512], F32, tag="hng")
                    nc.vector.tensor_copy(out=hng[:], in_=tp[:].rearrange("p a b -> p (a b)"))
                    Ag = spA.tile([128, 128], F32, tag="Ast")
                    nc.sync.dma_start(out=Ag[:], in_=Adr[g])
                    aps = ppB.tile([128, 512], F32, tag="psC")
                    nc.tensor.matmul(out=aps[:], lhsT=Ag[:], rhs=hng[:],
                                     start=True, stop=True)
                    asb = sp.tile([128, 512], F32, tag="agg_sb")
                    nc.vector.tensor_copy(out=asb[:], in_=aps[:])
                    tp2 = ppB.tile([128, 4, 128], F32, tag="psC")
                    zg = sp.tile([128, 4, 128], F32, tag="zg")
                    for m in range(4):
                        nc.tensor.transpose(out=tp2[:, m, :],
                                            in_=asb[:, m * 128:(m + 1) * 128],
                                            identity=ident[:])
                        nc.vector.tensor_scalar(
                            out=zg[:, m, :],
                            in0=h_fm[:, m, g * 128:(g + 1) * 128],
                            scalar1=float(ee), scalar2=None, op0=AL.mult)
                        nc.vector.tensor_tensor(out=zg[:, m, :], in0=zg[:, m, :],
                                                in1=tp2[:, m, :], op=AL.add)
                    nc.sync.dma_start(out=zdr[:, :, g * 128:(g + 1) * 128], in_=zg[:])
                mlp_two_layers(wm1, b1r, wm2, b2r, src_is_z0=False)
                scls, shfs = bn_stats_allreduce(zdr_chunk, BN_EPS, ginkey)
                for q in range(8):
                    zc = zdr_chunk(q)
                    for m in range(4):
                        nc.vector.tensor_scalar(out=zc[:, m, :], in0=zc[:, m, :],
                                                scalar1=scls[m][:], scalar2=shfs[m][:],
                                                op0=AL.mult, op1=AL.add)
                        nc.scalar.activation(out=zc[:, m, :], in_=zc[:, m, :],
                                             func=AF.Lrelu, alpha=LRELU_SLOPE)
                        nc.vector.tensor_tensor(out=h_fm[:, m, q * 512:(q + 1) * 512],
                                                in0=h_fm[:, m, q * 512:(q + 1) * 512],
                                                in1=zc[:, m, :], op=AL.add)
                scls, shfs = bn_stats_allreduce(hfm_chunk, BN_EPS * NP, bnskey)
                for m in range(4):
                    nc.vector.tensor_scalar(out=h_fm[:, m, :], in0=h_fm[:, m, :],
                                            scalar1=scls[m][:], scalar2=shfs[m][:],
                                            op0=AL.mult, op1=AL.add)

            # ---------------- readout ----------------
            h64 = big.tile([64, NODES_C], F32, tag="scr")
            l1w_sb = consts.tile([128, 4, 64], F32)
            nc.sync.dma_start(out=l1w_sb[:], in_=t_l1w[:].rearrange("(k p) o -> p k o", p=128))
            for q in range(8):
                ps64 = pp.tile([64, 512], F32, tag="psA")
                for k in range(4):
                    nc.tensor.matmul(out=ps64[:], lhsT=l1w_sb[:, k, :],
                                     rhs=h_fm[:, k, q * 512:(q + 1) * 512],
                                     start=(k == 0), stop=(k == 3))
                nc.scalar.activation(out=h64[:, q * 512:(q + 1) * 512], in_=ps64[:],
                                     func=AF.Lrelu, alpha=LRELU_SLOPE,
                                     bias=vc[:64, V_L1B, 0:1], scale=1.0)
            l2w_sb = consts.tile([64, 1], F32)
            nc.sync.dma_start(out=l2w_sb[:], in_=t_vecs[V_L2W, 0:64, None])
            hlast = rows.tile([1, NODES_C], F32, tag="rowB")
            b2t = consts.tile([1, 1], F32)
            nc.vector.memset(b2t[:], float(lin2b))
            for q in range(8):
                psl = pp.tile([1, 512], F32, tag="psA")
                nc.tensor.matmul(out=psl[:], lhsT=l2w_sb[:], rhs=h64[:, q * 512:(q + 1) * 512],
                                 start=True, stop=True)
                nc.scalar.activation(out=hlast[:, q * 512:(q + 1) * 512], in_=psl[:],
                                     func=AF.Lrelu, alpha=LRELU_SLOPE, bias=b2t[:], scale=1.0)

            # minmax -> p
            hg = hlast[:].rearrange("o (g i) -> o g i", g=GC)
            bmax = sp.tile([1, GC], F32, tag="bmax")
            bmin = sp.tile([1, GC], F32, tag="bmin")
            nc.vector.tensor_reduce(bmax[:], hg, AX.X, AL.max)
            nc.vector.tensor_reduce(bmin[:], hg, AX.X, AL.min)
            denom = sp.tile([1, GC], F32, tag="denom")
            nc.vector.tensor_scalar(out=denom[:], in0=bmax[:], scalar1=1e-6, scalar2=None,
                                    op0=AL.add)
            nc.vector.tensor_tensor(out=denom[:], in0=denom[:], in1=bmin[:], op=AL.subtract)
            nc.vector.reciprocal(out=denom[:], in_=denom[:])
            prow = rows.tile([1, NODES_C], F32, tag="rowA")
            pg = prow[:].rearrange("o (g i) -> o g i", g=GC)
            for g in range(GC):
                nc.vector.tensor_scalar(out=pg[:, g, :], in0=hg[:, g, :],
                                        scalar1=bmin[:, g:g + 1], scalar2=denom[:, g:g + 1],
                                        op0=AL.subtract, op1=AL.mult)
            nc.sync.dma_start(out=o_p[:].rearrange("(o n) -> o n", o=1), in_=prow[:])
            nc.sync.dma_start(out=hladr[:].rearrange("(o n) -> o n", o=1), in_=prow[:])
            p_col = sp.tile([128, GC], F32, tag="pcol")
            nc.sync.dma_start(out=p_col[:], in_=hladr[:].rearrange("(g i) -> i g", i=128))

            # ---------------- per-graph quantities + laplacians ----------------
            dW = sp.tile([128, GC], F32, tag="dW")
            Wp_ps = pp.tile([128, GC], F32, tag="psA")
            deg = sp.tile([128, GC], F32, tag="deg")
            for g in range(GC):
                Wg = spA.tile([128, 128], F32, tag="Wst")
                nc.sync.dma_start(out=Wg[:], in_=Wdr[g])
                nc.vector.tensor_reduce(dW[:, g:g + 1], Wg[:], AX.X, AL.add)
                nc.tensor.matmul(out=Wp_ps[:, g:g + 1], lhsT=Wg[:], rhs=p_col[:, g:g + 1],
                                 start=True, stop=True)
            q_t = sp.tile([128, GC], F32, tag="qt")
            nc.vector.tensor_tensor(out=q_t[:], in0=dW[:], in1=p_col[:], op=AL.mult)
            nc.vector.tensor_tensor(out=q_t[:], in0=q_t[:], in1=Wp_ps[:], op=AL.subtract)
            nc.vector.tensor_tensor(out=q_t[:], in0=q_t[:], in1=p_col[:], op=AL.mult)
            pair_ps = pp.tile([1, GC], F32, tag="psA")
            nc.tensor.matmul(out=pair_ps[:], lhsT=ones_c[:], rhs=q_t[:], start=True, stop=True)
            gam_ps = pp.tile([1, GC], F32, tag="psB")
            nc.tensor.matmul(out=gam_ps[:], lhsT=ones_c[:], rhs=dW[:], start=True, stop=True)
            gamma = sp.tile([1, GC], F32, tag="gamma")
            nc.vector.tensor_scalar(out=gamma[:], in0=gam_ps[:], scalar1=0.5, scalar2=None,
                                    op0=AL.mult)
            frel = sp.tile([1, GC], F32, tag="frel")
            nc.vector.tensor_scalar(out=frel[:], in0=pair_ps[:], scalar1=-1.0,
                                    scalar2=None, op0=AL.mult)
            nc.vector.tensor_tensor(out=frel[:], in0=gamma[:], in1=frel[:], op=AL.add)
            nc.sync.dma_start(out=o_fr[:].rearrange("(o n) -> o n", o=1), in_=frel[:])

            omp = sp.tile([128, GC], F32, tag="omp")
            nc.vector.tensor_scalar(out=omp[:], in0=p_col[:], scalar1=-1.0, scalar2=1.0,
                                    op0=AL.mult, op1=AL.add)
            for g in range(GC):
                Ag = spA.tile([128, 128], F32, tag="Ast")
                nc.sync.dma_start(out=Ag[:], in_=Adr[g])
                nc.vector.tensor_reduce(deg[:, g:g + 1], Ag[:], AX.X, AL.add)
                l0 = sp.tile([128, 128], F32, tag="l0")
                nc.vector.tensor_scalar(out=l0[:], in0=ident[:], scalar1=deg[:, g:g + 1],
                                        scalar2=None, op0=AL.mult)
                nc.vector.tensor_tensor(out=l0[:], in0=l0[:], in1=Ag[:], op=AL.subtract)
                nc.sync.dma_start(out=lapdr[g].rearrange("(i j) -> i j", i=NP),
                                  in_=l0[:])
                pb_ps = ppB.tile([128, 128], F32, tag="psC")
                nc.tensor.matmul(out=pb_ps[:], lhsT=ones_r[:], rhs=pg[:, g, :],
                                 start=True, stop=True)
                bm = sp.tile([128, 128], F32, tag="bm")
                nc.vector.tensor_scalar(out=bm[:], in0=pb_ps[:], scalar1=omp[:, g:g + 1],
                                        scalar2=None, op0=AL.subtract)
                nc.scalar.activation(out=bm[:], in_=bm[:], func=AF.Square)
                nc.vector.tensor_tensor(out=bm[:], in0=bm[:], in1=Ag[:], op=AL.mult)
                degn = sp.tile([128, 1], F32, tag="degn")
                nc.vector.tensor_reduce(degn[:], bm[:], AX.X, AL.add)
                ln_ = sp.tile([128, 128], F32, tag="ln_")
                nc.vector.tensor_scalar(out=ln_[:], in0=ident[:], scalar1=degn[:],
                                        scalar2=None, op0=AL.mult)
                nc.vector.tensor_tensor(out=ln_[:], in0=ln_[:], in1=bm[:], op=AL.subtract)
                nc.sync.dma_start(out=lapdr[GC + g].rearrange("(i j) -> i j", i=NP),
                                  in_=ln_[:])

            # ---------------- Householder tridiagonalization ----------------
            M = 2 * GC
            L = big.tile([M, NP, NP], F32, tag="h_fm")
            nc.sync.dma_start(out=L[:], in_=lapdr[:].rearrange("m (i j) -> m i j", i=NP))
            nc.sync.dma_start(out=o_lap[:].rearrange("m (i j) -> m i j", i=NP), in_=L[:])
            tmp = big.tile([M, NP, 64], F32, tag="scr")
            alphas = consts.tile([M, NP], F32)
            betasq = consts.tile([M, NP], F32)
            nc.vector.memset(betasq[:], 1e-30)
            tiny1 = consts.tile([M, 1], F32)
            nc.vector.memset(tiny1[:], 1e-30)
            onesM = consts.tile([M, 1], F32)
            nc.vector.memset(onesM[:], 1.0)

            for k in range(HH_STEPS):
                r = NP - k - 1
                nc.vector.tensor_copy(out=alphas[:, k:k + 1], in_=L[:, k, k:k + 1])
                xk = sp.tile([M, NP], F32, tag="xk")
                nc.vector.tensor_copy(out=xk[:, :r], in_=L[:, k + 1:, k])
                sq = sp.tile([M, NP], F32, tag="sqk")
                nc.vector.tensor_tensor(out=sq[:, :r], in0=xk[:, :r], in1=xk[:, :r], op=AL.mult)
                nsq = sp.tile([M, 1], F32, tag="nsq")
                nc.vector.tensor_reduce(nsq[:], sq[:, :r], AX.X, AL.add)
                nc.vector.tensor_scalar(out=betasq[:, k:k + 1], in0=nsq[:], scalar1=tiny1[:],
                                        scalar2=None, op0=AL.max)
                nrm = sp.tile([M, 1], F32, tag="nrm")
                nc.scalar.activation(out=nrm[:], in_=nsq[:], func=AF.Sqrt)
                sg = sp.tile([M, 1], F32, tag="sg")
                nc.scalar.activation(out=sg[:], in_=xk[:, 0:1], func=AF.Sign)
                sgz = sp.tile([M, 1], U32, tag="sgz")
                nc.vector.tensor_scalar(out=sgz[:], in0=sg[:], scalar1=0.0, scalar2=None,
                                        op0=AL.is_equal)
                nc.vector.copy_predicated(sg[:], sgz[:], onesM[:])
                snr = sp.tile([M, 1], F32, tag="snr")
                nc.vector.tensor_tensor(out=snr[:], in0=sg[:], in1=nrm[:], op=AL.mult)
                nc.vector.tensor_tensor(out=xk[:, 0:1], in0=xk[:, 0:1], in1=snr[:], op=AL.add)
                nc.vector.tensor_tensor(out=sq[:, :r], in0=xk[:, :r], in1=xk[:, :r], op=AL.mult)
                vns = sp.tile([M, 1], F32, tag="vns")
                nc.vector.tensor_reduce(vns[:], sq[:, :r], AX.X, AL.add)
                nc.vector.tensor_scalar(out=vns[:], in0=vns[:], scalar1=tiny1[:],
                                        scalar2=None, op0=AL.max)
                tau = sp.tile([M, 1], F32, tag="tau")
                nc.vector.reciprocal(out=tau[:], in_=vns[:])
                nc.vector.tensor_scalar(out=tau[:], in0=tau[:], scalar1=2.0, scalar2=None,
                                        op0=AL.mult)
                ztau = sp.tile([M, 1], U32, tag="ztau")
                nc.vector.tensor_scalar(out=ztau[:], in0=nsq[:], scalar1=1e-20, scalar2=None,
                                        op0=AL.is_lt)
                zz = sp.tile([M, 1], F32, tag="zz")
                nc.vector.memset(zz[:], 0.0)
                nc.vector.copy_predicated(tau[:], ztau[:], zz[:])
                Lsub = L[:, k + 1:, k + 1:]
                wk = sp.tile([M, NP], F32, tag="wk")
                jsp = (r + 1) // 2
                for (j0, j1) in ((0, jsp), (jsp, r)):
                    jw = j1 - j0
                    if jw <= 0:
                        continue
                    nc.vector.tensor_tensor(
                        out=tmp[:, :r, :jw],
                        in0=L[:, k + 1:, k + 1 + j0:k + 1 + j1],
                        in1=xk[:, :r, None].broadcast_to([M, r, jw]),
                        op=AL.mult)
                    tTh = bass.AP(tensor=tmp.tensor, offset=tmp[:].offset,
                                  ap=[tmp[:].ap[0], [1, jw], [64, r]])
                    nc.vector.tensor_reduce(wk[:, j0:j1], tTh, AX.X, AL.add)
                nc.vector.tensor_tensor(out=sq[:, :r], in0=xk[:, :r], in1=wk[:, :r], op=AL.mult)
                Kd = sp.tile([M, 1], F32, tag="Kd")
                nc.vector.tensor_reduce(Kd[:], sq[:, :r], AX.X, AL.add)
                c2 = sp.tile([M, 1], F32, tag="c2")
                nc.vector.tensor_tensor(out=c2[:], in0=tau[:], in1=tau[:], op=AL.mult)
                nc.vector.tensor_tensor(out=c2[:], in0=c2[:], in1=Kd[:], op=AL.mult)
                nc.vector.tensor_scalar(out=c2[:], in0=c2[:], scalar1=0.5, scalar2=None,
                                        op0=AL.mult)
                uk = sp.tile([M, NP], F32, tag="uk")
                nc.vector.tensor_scalar(out=uk[:, :r], in0=wk[:, :r], scalar1=tau[:],
                                        scalar2=None, op0=AL.mult)
                nc.vector.tensor_scalar(out=sq[:, :r], in0=xk[:, :r], scalar1=c2[:],
                                        scalar2=None, op0=AL.mult)
                nc.vector.tensor_tensor(out=uk[:, :r], in0=uk[:, :r], in1=sq[:, :r],
                                        op=AL.subtract)
                for (j0, j1) in ((0, jsp), (jsp, r)):
                    jw = j1 - j0
                    if jw <= 0:
                        continue
                    Lh = L[:, k + 1:, k + 1 + j0:k + 1 + j1]
                    nc.vector.tensor_tensor(
                        out=tmp[:, :r, :jw],
                        in0=xk[:, :r, None].broadcast_to([M, r, jw]),
                        in1=uk[:, None, j0:j1].broadcast_to([M, r, jw]),
                        op=AL.mult)
                    nc.vector.tensor_tensor(out=Lh, in0=Lh, in1=tmp[:, :r, :jw],
                                            op=AL.subtract)
                    nc.vector.tensor_tensor(
                        out=tmp[:, :r, :jw],
                        in0=uk[:, :r, None].broadcast_to([M, r, jw]),
                        in1=xk[:, None, j0:j1].broadcast_to([M, r, jw]),
                        op=AL.mult)
                    nc.vector.tensor_tensor(out=Lh, in0=Lh, in1=tmp[:, :r, :jw],
                                            op=AL.subtract)

            nc.vector.tensor_copy(out=alphas[:, NP - 2:NP - 1], in_=L[:, NP - 2, NP - 2:NP - 1])
            nc.vector.tensor_copy(out=alphas[:, NP - 1:NP], in_=L[:, NP - 1, NP - 1:NP])
            lastb = sp.tile([M, 1], F32, tag="lastb")
            nc.vector.tensor_tensor(out=lastb[:], in0=L[:, NP - 1, NP - 2:NP - 1],
                                    in1=L[:, NP - 1, NP - 2:NP - 1], op=AL.mult)
            nc.vector.tensor_scalar(out=betasq[:, NP - 2:NP - 1], in0=lastb[:],
                                    scalar1=tiny1[:], scalar2=None, op0=AL.max)

            # ---------------- Sturm multisection ----------------
            nc.sync.dma_start(out=abdr[0], in_=alphas[:])
            nc.sync.dma_start(out=abdr[1], in_=betasq[:])
            al_b = consts.tile([NPROB, NP], F32)
            bs_b = consts.tile([NPROB, NP], F32)
            nc.sync.dma_start(out=al_b[:M, :], in_=abdr[0])
            nc.sync.dma_start(out=al_b[M:, :], in_=abdr[0, GC:, :])
            nc.sync.dma_start(out=bs_b[:M, :], in_=abdr[1])
            nc.sync.dma_start(out=bs_b[M:, :], in_=abdr[1, GC:, :])
            tgt = consts.tile([NPROB, 1], F32)
            nc.vector.memset(tgt[:GC], 3.0)
            nc.vector.memset(tgt[GC:M], 2.0)
            nc.vector.memset(tgt[M:], 3.0)
            amax = sp.tile([NPROB, 1], F32, tag="amax")
            nc.vector.tensor_reduce(amax[:], al_b[:], AX.X, AL.max)
            bmx = sp.tile([NPROB, 1], F32, tag="bmx")
            nc.vector.tensor_reduce(bmx[:], bs_b[:], AX.X, AL.max)
            nc.scalar.activation(out=bmx[:], in_=bmx[:], func=AF.Sqrt)
            hi = sp.tile([NPROB, 1], F32, tag="hi")
            nc.vector.tensor_scalar(out=hi[:], in0=bmx[:], scalar1=2.0, scalar2=None,
                                    op0=AL.mult)
            nc.vector.tensor_tensor(out=hi[:], in0=hi[:], in1=amax[:], op=AL.add)
            nc.vector.tensor_scalar(out=hi[:], in0=hi[:], scalar1=1.001, scalar2=0.01,
                                    op0=AL.mult, op1=AL.add)
            lo = sp.tile([NPROB, 1], F32, tag="lo")
            nc.vector.tensor_scalar(out=lo[:], in0=hi[:], scalar1=-0.01, scalar2=None,
                                    op0=AL.mult)
            iotaS = consts.tile([NPROB, BIS_S], F32)
            nc.gpsimd.iota(iotaS[:], pattern=[[1, BIS_S]], base=1, channel_multiplier=0,
                           allow_small_or_imprecise_dtypes=True)
            estore = big.tile([NPROB, NP, BIS_S], F32, tag="scr")
            negb = consts.tile([NPROB, NP], F32)
            nc.vector.tensor_scalar(out=negb[:], in0=bs_b[:], scalar1=-1.0, scalar2=None,
                                    op0=AL.mult)
            nega = consts.tile([NPROB, NP], F32)
            nc.vector.tensor_scalar(out=nega[:], in0=al_b[:], scalar1=-1.0, scalar2=None,
                                    op0=AL.mult)

            for _ in range(BIS_PASSES):
                stepw = sp.tile([NPROB, 1], F32, tag="stepw")
                nc.vector.tensor_tensor(out=stepw[:], in0=hi[:], in1=lo[:], op=AL.subtract)
                nc.vector.tensor_scalar(out=stepw[:], in0=stepw[:],
                                        scalar1=float(1.0 / (BIS_S + 1)), scalar2=None,
                                        op0=AL.mult)
                sig = sp.tile([NPROB, BIS_S], F32, tag="sig")
                nc.vector.tensor_scalar(out=sig[:], in0=iotaS[:], scalar1=stepw[:],
                                        scalar2=lo[:], op0=AL.mult, op1=AL.add)
                nc.vector.tensor_scalar(out=estore[:, 0, :], in0=sig[:],
                                        scalar1=al_b[:, 0:1], scalar2=None, op0=AL.subtract)
                for i in range(1, NP):
                    rec = sp.tile([NPROB, BIS_S], F32, tag="rec")
                    nc.vector.reciprocal(out=rec[:], in_=estore[:, i - 1, :])
                    nc.vector.tensor_scalar(out=estore[:, i, :], in0=rec[:],
                                            scalar1=negb[:, i - 1:i], scalar2=nega[:, i:i + 1],
                                            op0=AL.mult, op1=AL.add)
                    nc.vector.tensor_tensor(out=estore[:, i, :], in0=estore[:, i, :],
                                            in1=sig[:], op=AL.add)
                nc.vector.tensor_scalar(out=estore[:], in0=estore[:], scalar1=0.0,
                                        scalar2=None, op0=AL.is_gt)
                cT = bass.AP(tensor=estore.tensor, offset=estore[:].offset,
                             ap=[estore[:].ap[0], [1, BIS_S], [BIS_S, NP]])
                cnt = sp.tile([NPROB, BIS_S], F32, tag="cnt")
                nc.vector.tensor_reduce(cnt[:], cT, AX.X, AL.add)
                meets = sp.tile([NPROB, BIS_S], F32, tag="meets")
                nc.vector.tensor_scalar(out=meets[:], in0=cnt[:], scalar1=tgt[:],
                                        scalar2=None, op0=AL.is_ge)
                selt = sp.tile([NPROB, BIS_S], F32, tag="selt")
                nc.vector.tensor_scalar(out=selt[:], in0=meets[:], scalar1=-1e6,
                                        scalar2=1e6, op0=AL.mult, op1=AL.add)
                nc.vector.tensor_tensor(out=selt[:], in0=selt[:], in1=iotaS[:], op=AL.add)
                sstar = sp.tile([NPROB, 1], F32, tag="sstar")
                nc.vector.tensor_reduce(sstar[:], selt[:], AX.X, AL.min)
                smax = sp.tile([NPROB, 1], F32, tag="smax")
                nc.vector.memset(smax[:], float(BIS_S + 1))
                nc.vector.tensor_tensor(out=sstar[:], in0=sstar[:], in1=smax[:], op=AL.min)
                nc.vector.tensor_scalar(out=sstar[:], in0=sstar[:], scalar1=-1.0,
                                        scalar2=None, op0=AL.add)
                nc.vector.tensor_scalar(out=sstar[:], in0=sstar[:], scalar1=stepw[:],
                                        scalar2=None, op0=AL.mult)
                nc.vector.tensor_tensor(out=lo[:], in0=lo[:], in1=sstar[:], op=AL.add)
                nc.vector.tensor_tensor(out=hi[:], in0=lo[:], in1=stepw[:], op=AL.add)

            lam = sp.tile([NPROB, 1], F32, tag="lam")
            nc.vector.tensor_tensor(out=lam[:], in0=lo[:], in1=hi[:], op=AL.add)
            nc.vector.tensor_scalar(out=lam[:], in0=lam[:], scalar1=0.5, scalar2=None,
                                    op0=AL.mult)
            nc.sync.dma_start(out=lamdr[:, None], in_=lam[:])
            lrow = rows.tile([1, NPROB], F32, tag="rowC")
            nc.sync.dma_start(out=lrow[:], in_=lamdr[:].rearrange("(o n) -> o n", o=1))
            nc.sync.dma_start(out=o_ev[:].rearrange("(o n) -> o n", o=1), in_=lrow[:])
            nc.sync.dma_start(out=o_ab[0], in_=alphas[:])
            nc.sync.dma_start(out=o_ab[1], in_=betasq[:])

            aaa = sp.tile([1, GC], F32, tag="aaa")
            nc.vector.reciprocal(out=aaa[:], in_=lrow[:, 0:GC])
            nc.vector.tensor_scalar(out=aaa[:], in0=aaa[:], scalar1=float(-np.log(1e-4)),
                                    scalar2=None, op0=AL.mult)
            d21 = sp.tile([1, GC], F32, tag="d21")
            nc.vector.tensor_tensor(out=d21[:], in0=lrow[:, M:], in1=lrow[:, GC:M],
                                    op=AL.subtract)
            nc.vector.tensor_tensor(out=d21[:], in0=d21[:], in1=aaa[:], op=AL.mult)
            nc.vector.tensor_scalar(out=d21[:], in0=d21[:], scalar1=-1.0, scalar2=None,
                                    op0=AL.mult)
            grel = sp.tile([1, GC], F32, tag="grel")
            nc.scalar.activation(out=grel[:], in_=d21[:], func=AF.Exp)
            nc.sync.dma_start(out=o_gr[:].rearrange("(o n) -> o n", o=1), in_=grel[:])
            lossr = sp.tile([1, GC], F32, tag="lossr")
            nc.vector.tensor_tensor(out=lossr[:], in0=gamma[:], in1=grel[:], op=AL.mult)
            fterm = sp.tile([1, GC], F32, tag="fterm")
            nc.vector.tensor_scalar(out=fterm[:], in0=frel[:], scalar1=float(120.0 / 36.0),
                                    scalar2=None, op0=AL.mult)
            nc.vector.tensor_tensor(out=lossr[:], in0=lossr[:], in1=fterm[:], op=AL.add)
            nc.sync.dma_start(out=o_loss[:].rearrange("(o n) -> o n", o=1), in_=lossr[:])

    split_waits(nc)
    return nc


def kernel(x, edge_attr, params, edge_index, batch, edge_batch_index):
    import sys, os
    sys.path.insert(0, os.path.dirname(os.path.abspath(__file__)))
    from concourse.bass_utils import run_bass_kernel_spmd

    f32 = np.float32
    x = np.asarray(x, f32)
    ea = np.asarray(edge_attr, f32)
    ei = np.asarray(edge_index).astype(np.int64)
    batch_np = np.asarray(batch).astype(np.int64)

    counts = np.bincount(batch_np, minlength=G)
    uniform = bool((counts == NP).all())

    row = ei[0]
    E = G * E_HALF
    src, dst = row[:E], ei[1][:E]
    g_of = src // NP
    lr = (src - g_of * NP).astype(f32)
    lc = (dst - g_of * NP).astype(f32)
    w_half = ea[:E].astype(f32)

    def P(t):
        return np.asarray(t, f32)

    pr = params
    consts_in = (
        1.0 + float(np.asarray(pr["conv1"]["eps"])),
        1.0 + float(np.asarray(pr["convs"][0]["eps"])),
        1.0 + float(np.asarray(pr["convs"][1]["eps"])),
        float(np.asarray(pr["lin2"]["b"]).reshape(-1)[0]),
    )

    Wm = np.stack([
        P(pr["conv1"]["l2"]["w"]),
        P(pr["convs"][0]["l1"]["w"]), P(pr["convs"][0]["l2"]["w"]),
        P(pr["convs"][1]["l1"]["w"]), P(pr["convs"][1]["l2"]["w"]),
    ])
    vecs = np.zeros((21, D), f32)
    vecs[0] = P(pr["conv1"]["l1"]["w"]).reshape(-1)
    vecs[1] = P(pr["conv1"]["l1"]["b"]); vecs[2] = P(pr["conv1"]["l2"]["b"])
    vecs[3] = P(pr["convs"][0]["l1"]["b"]); vecs[4] = P(pr["convs"][0]["l2"]["b"])
    vecs[5] = P(pr["convs"][1]["l1"]["b"]); vecs[6] = P(pr["convs"][1]["l2"]["b"])
    vecs[7] = P(pr["conv1"]["bn_g"]); vecs[8] = P(pr["conv1"]["bn_b"])
    vecs[9] = P(pr["bn1_g"]); vecs[10] = P(pr["bn1_b"])
    vecs[11] = P(pr["convs"][0]["bn_g"]); vecs[12] = P(pr["convs"][0]["bn_b"])
    vecs[13] = P(pr["bns"][0]["g"]); vecs[14] = P(pr["bns"][0]["b"])
    vecs[15] = P(pr["convs"][1]["bn_g"]); vecs[16] = P(pr["convs"][1]["bn_b"])
    vecs[17] = P(pr["bns"][1]["g"]); vecs[18] = P(pr["bns"][1]["b"])
    vecs[19, :64] = P(pr["lin1"]["b"])
    vecs[20, :64] = P(pr["lin2"]["w"]).reshape(-1)

    key = ("prog", consts_in, uniform)
    if key not in _cache:
        _cache[key] = _build_program(consts_in, uniform)
    nc = _cache[key]

    in_maps = []
    for c in range(NCORES):
        gs = c * GC
        es = gs * E_HALF
        ns = gs * NP
        in_maps.append({
            "lr": lr[es:es + EC], "lc": lc[es:es + EC], "ew": w_half[es:es + EC],
            "x0": x[ns:ns + NODES_C],
            "Wm": Wm, "lin1w": P(pr["lin1"]["w"]), "vecs": vecs,
        })
    res = run_bass_kernel_spmd(nc, in_maps, core_ids=list(range(NCORES)))
    p_full = np.concatenate([res.results[c]["p_out"] for c in range(NCORES)])
    loss = np.concatenate([res.results[c]["loss"] for c in range(NCORES)])
    f_relax = np.concatenate([res.results[c]["f_relax"] for c in range(NCORES)])
    g_relax = np.concatenate([res.results[c]["g_relax"] for c in range(NCORES)])
    loss_mean = np.float32(loss.mean())
    return (p_full.astype(f32), loss.astype(f32), loss_mean,
            f_relax.astype(f32), g_relax.astype(f32))


# revision 14
# speedup vs baseline: 397.9519x; 397.9519x over previous
"""Trainium2 Bass kernel for nn_ELECT_Mnist (GIN message passing + ELECT loss).

Strategy (8 NeuronCores, data-parallel over graphs, 32 graphs/core):
 - Dense per-graph adjacency built on device via one-hot matmuls (edges are
   graph-local), so GIN aggregation becomes one [128,128]@[128,512] matmul
   per graph instead of irregular gather/scatter.
 - GIN MLPs run feature-major on the tensor engine; BatchNorm batch stats
   are global over all 32768 nodes -> per-core partials + AllReduce.
 - GraphSizeNorm (x / sqrt(128)) is folded into the following BatchNorm
   exactly (uniform graph size), by scaling the BN eps by 128.
 - Spectral part: per-graph 128x128 Laplacian eigenvalues via batched
   Householder tridiagonalization (graph-per-partition layout) + multi-
   section Sturm bisection, fully on device.
Self-contained: hardcodes shapes from the problem spec.
"""

import numpy as np

G, NP, E_HALF = 256, 128, 1024
D = 512
NCORES = 8
GC = G // NCORES              # 32 graphs per core
NODES_C = GC * NP             # 4096 nodes per core
HALF_N = NODES_C // 2         # 2048
EC = GC * E_HALF              # 32768 (undirected-half) edges per core
N = G * NP
BN_EPS = 1e-5
LRELU_SLOPE = 0.01
HH_STEPS = NP - 2
BIS_S = 16
BIS_PASSES = 5
NPROB = 3 * GC                # 96 bisection problems per core

_cache = {}


def _split_waits(nc, maxw=1):
    """This compiler build allows only ONE semaphore wait per instruction;
    split excess waits onto same-engine nops inserted just before."""
    import concourse.mybir as mybir
    for f in nc.m.functions:
        for bb in f.blocks:
            insts = bb.instructions
            if not any(
                ins.sync_info and ins.sync_info.on_wait
                and len(ins.sync_info.on_wait) > maxw
                for ins in insts
            ):
                continue
            new_list = []
            for ins in insts:
                si = ins.sync_info
                w = list(si.on_wait) if si and si.on_wait else []
                if len(w) > maxw:
                    extra, keep = w[:-maxw], w[-maxw:]
                    for j in range(0, len(extra), maxw):
                        chunk = extra[j:j + maxw]
                        nop = nc.engines[ins.engine].nop(nofuse=True, hint="waitsplit")
                        nop_ins = nop.ins
                        for bb2 in f.blocks:
                            lst = bb2.instructions
                            if lst and lst[-1] is nop_ins:
                                lst.pop()
                                break
                        nop_ins.sync_info = mybir.SyncInfo(on_wait=chunk, on_update=[])
                        new_list.append(nop_ins)
                    ins.sync_info = mybir.SyncInfo(
                        on_wait=keep,
                        on_update=list(si.on_update) if si.on_update else [])
                new_list.append(ins)
            insts[:] = new_list


def _build_program(consts_in, uniform_counts):
    import concourse.bass as bass
    import concourse.mybir as mybir
    import concourse.tile as tile
    from concourse.masks import make_identity

    F32 = mybir.dt.float32
    U32 = mybir.dt.uint32
    AL = mybir.AluOpType
    AF = mybir.ActivationFunctionType
    AX = mybir.AxisListType

    assert uniform_counts, "non-uniform graph sizes not supported by this kernel"
    e1, e2, e3, lin2b = consts_in

    nc = bass.Bass(num_devices=NCORES)

    t_lr = nc.dram_tensor("lr", [EC], F32, kind="ExternalInput")
    t_lc = nc.dram_tensor("lc", [EC], F32, kind="ExternalInput")
    t_ew = nc.dram_tensor("ew", [EC], F32, kind="ExternalInput")
    t_x = nc.dram_tensor("x0", [NODES_C], F32, kind="ExternalInput")
    t_Wm = nc.dram_tensor("Wm", [5, D, D], F32, kind="ExternalInput")
    t_l1w = nc.dram_tensor("lin1w", [D, 64], F32, kind="ExternalInput")
    t_vecs = nc.dram_tensor("vecs", [21, D], F32, kind="ExternalInput")
    o_p = nc.dram_tensor("p_out", [NODES_C], F32, kind="ExternalOutput")
    o_loss = nc.dram_tensor("loss", [GC], F32, kind="ExternalOutput")
    o_fr = nc.dram_tensor("f_relax", [GC], F32, kind="ExternalOutput")
    o_gr = nc.dram_tensor("g_relax", [GC], F32, kind="ExternalOutput")

    V_W1, V_B1, V_B2 = 0, 1, 2
    V_C0B1, V_C0B2, V_C1B1, V_C1B2 = 3, 4, 5, 6
    V_GBN = {"gin1": (7, 8), "bn1": (9, 10), "gin2": (11, 12),
             "bns0": (13, 14), "gin3": (15, 16), "bns1": (17, 18)}
    V_L1B, V_L2W = 19, 20

    with tile.TileContext(nc) as tc:
        import contextlib
        ctx = contextlib.ExitStack()
        with ctx:
            consts = ctx.enter_context(tc.tile_pool(name="consts", bufs=1))
            dram = ctx.enter_context(tc.tile_pool(name="dram", bufs=1, space="DRAM"))
            big = ctx.enter_context(tc.tile_pool(name="big", bufs=1))
            sp = ctx.enter_context(tc.tile_pool(name="sp", bufs=2))
            mid = ctx.enter_context(tc.tile_pool(name="mid", bufs=1))
            rows = ctx.enter_context(tc.tile_pool(name="rows", bufs=1))
            spA = ctx.enter_context(tc.tile_pool(name="spA", bufs=2))
            pp = ctx.enter_context(tc.tile_pool(name="pp", bufs=2, space="PSUM"))
            ppB = ctx.enter_context(tc.tile_pool(name="ppB", bufs=2, space="PSUM"))
            ppF = ctx.enter_context(tc.tile_pool(name="ppF", bufs=1, space="PSUM"))

            ident = consts.tile([128, 128], F32)
            make_identity(nc, ident[:])
            iota_t = consts.tile([128, 128], F32)
            nc.gpsimd.iota(iota_t[:], pattern=[[1, 128]], base=0,
                           channel_multiplier=0, allow_small_or_imprecise_dtypes=True)
            ones_c = consts.tile([128, 1], F32)
            nc.vector.memset(ones_c[:], 1.0)
            ones_r = consts.tile([1, 128], F32)
            nc.vector.memset(ones_r[:], 1.0)
            vc = consts.tile([128, 21, 4], F32)
            nc.sync.dma_start(out=vc[:], in_=t_vecs[:].rearrange("v (m p) -> p v m", p=128))

            Adr = dram.tile([GC, NP, NP], F32)
            Wdr = dram.tile([GC, NP, NP], F32)
            lapdr = dram.tile([2 * GC, NP * NP], F32)
            z0dr = dram.tile([NODES_C], F32)
            hladr = dram.tile([NODES_C], F32)
            lamdr = dram.tile([NPROB], F32)
            abdr = dram.tile([2, 2 * GC, NP], F32)
            bn_in = dram.tile([128, 8], F32)
            bn_outs = [dram.tile([128, 8], F32, addr_space="Shared",
                                 name=f"bn_out{i}", tag=f"bn_out{i}")
                       for i in range(6)]
            bn_ctr = [0]

            # ---------------- Phase A: adjacency build ----------------
            lr_sb = consts.tile([128, GC, 8], F32)
            lc_sb = consts.tile([128, GC, 8], F32)
            ew_sb = consts.tile([128, GC, 8], F32)
            nc.sync.dma_start(out=lr_sb[:], in_=t_lr[:].rearrange("(g t p) -> p g t", p=128, t=8))
            nc.sync.dma_start(out=lc_sb[:], in_=t_lc[:].rearrange("(g t p) -> p g t", p=128, t=8))
            nc.sync.dma_start(out=ew_sb[:], in_=t_ew[:].rearrange("(g t p) -> p g t", p=128, t=8))

            for g in range(GC):
                Cp = pp.tile([128, 128], F32, tag="psA")
                Cwp = pp.tile([128, 128], F32, tag="psB")
                for t in range(8):
                    ohr = spA.tile([128, 128], F32, tag="ohr")
                    ohc = spA.tile([128, 128], F32, tag="ohc")
                    ohrw = spA.tile([128, 128], F32, tag="ohrw")
                    nc.vector.tensor_scalar(out=ohr[:], in0=iota_t[:],
                                            scalar1=lr_sb[:, g, t:t + 1], scalar2=None,
                                            op0=AL.is_equal)
                    nc.vector.tensor_scalar(out=ohc[:], in0=iota_t[:],
                                            scalar1=lc_sb[:, g, t:t + 1], scalar2=None,
                                            op0=AL.is_equal)
                    nc.vector.tensor_scalar(out=ohrw[:], in0=ohr[:],
                                            scalar1=ew_sb[:, g, t:t + 1], scalar2=None,
                                            op0=AL.mult)
                    nc.tensor.matmul(out=Cp[:], lhsT=ohr[:], rhs=ohc[:],
                                     start=(t == 0), stop=(t == 7))
                    nc.tensor.matmul(out=Cwp[:], lhsT=ohrw[:], rhs=ohc[:],
                                     start=(t == 0), stop=(t == 7))
                for (P_, dst) in ((Cp, Adr), (Cwp, Wdr)):
                    Cs = sp.tile([128, 128], F32, tag="Cs")
                    nc.vector.tensor_copy(out=Cs[:], in_=P_[:])
                    Tp = pp.tile([128, 128], F32, tag="psA")
                    nc.tensor.transpose(out=Tp[:], in_=Cs[:], identity=ident[:])
                    As = sp.tile([128, 128], F32, tag="As")
                    nc.vector.tensor_tensor(out=As[:], in0=Cs[:], in1=Tp[:], op=AL.add)
                    nc.sync.dma_start(out=dst[g], in_=As[:])

            # ---------------- persistent GNN tiles ----------------
            h_fm = big.tile([128, 4, NODES_C], F32, tag="h_fm")   # 8MB
            zdr = dram.tile([128, 4, NODES_C], F32)               # z mirror in DRAM

            def bn_stats_allreduce(load_chunk, eps_eff, key):
                """load_chunk(q) -> [128,4,512] AP for node-chunk q. Returns
                per-chunk (scl[m], shf[m]) tiles after global AllReduce."""
                gi, bi = V_GBN[key]
                stats = sp.tile([128, 8], F32, tag="bnstats")
                nc.vector.memset(stats[:], 0.0)
                for q in range(8):
                    zc = load_chunk(q)
                    for m in range(4):
                        part = sp.tile([128, 1], F32, tag="bnpart")
                        nc.vector.tensor_reduce(part[:], zc[:, m, :], AX.X, AL.add)
                        nc.vector.tensor_tensor(out=stats[:, 2 * m:2 * m + 1],
                                                in0=stats[:, 2 * m:2 * m + 1],
                                                in1=part[:], op=AL.add)
                        part2 = sp.tile([128, 1], F32, tag="bnpart2")
                        sqs = mid.tile([128, 512], F32, tag="sqs")
                        nc.scalar.activation(out=sqs[:], in_=zc[:, m, :], func=AF.Square)
                        nc.vector.tensor_reduce(part2[:], sqs[:], AX.X, AL.add)
                        nc.vector.tensor_tensor(out=stats[:, 2 * m + 1:2 * m + 2],
                                                in0=stats[:, 2 * m + 1:2 * m + 2],
                                                in1=part2[:], op=AL.add)
                nc.sync.dma_start(out=bn_in[:], in_=stats[:])
                bno = bn_outs[bn_ctr[0]]; bn_ctr[0] += 1
                nc.gpsimd.collective_compute(
                    "AllReduce", AL.add, replica_groups=[list(range(NCORES))],
                    ins=[bn_in[:].opt()], outs=[bno[:].opt()])
                gstats = sp.tile([128, 8], F32, tag="bngst")
                nc.sync.dma_start(out=gstats[:], in_=bno[:])
                epst = sp.tile([128, 1], F32, tag="bneps")
                nc.vector.memset(epst[:], float(eps_eff))
                scls, shfs = [], []
                for m in range(4):
                    mu = sp.tile([128, 1], F32, tag=f"bnmu{m}")
                    nc.vector.tensor_scalar(out=mu[:], in0=gstats[:, 2 * m:2 * m + 1],
                                            scalar1=float(1.0 / N), scalar2=None, op0=AL.mult)
                    var = sp.tile([128, 1], F32, tag=f"bnvar{m}")
                    nc.vector.tensor_scalar(out=var[:], in0=gstats[:, 2 * m + 1:2 * m + 2],
                                            scalar1=float(1.0 / N), scalar2=None, op0=AL.mult)
                    mu2 = sp.tile([128, 1], F32, tag=f"bnmu2{m}")
                    nc.vector.tensor_tensor(out=mu2[:], in0=mu[:], in1=mu[:], op=AL.mult)
                    nc.vector.tensor_tensor(out=var[:], in0=var[:], in1=mu2[:], op=AL.subtract)
                    rstd = sp.tile([128, 1], F32, tag=f"bnrstd{m}")
                    nc.scalar.activation(out=rstd[:], in_=var[:], func=AF.Sqrt,
                                         bias=epst[:], scale=1.0)
                    nc.vector.reciprocal(out=rstd[:], in_=rstd[:])
                    scl = sp.tile([128, 1], F32, tag=f"bnscl{m}")
                    nc.vector.tensor_tensor(out=scl[:], in0=rstd[:], in1=vc[:, gi, m:m + 1],
                                            op=AL.mult)
                    shf = sp.tile([128, 1], F32, tag=f"bnshf{m}")
                    nc.vector.tensor_tensor(out=shf[:], in0=mu[:], in1=scl[:], op=AL.mult)
                    nc.vector.tensor_tensor(out=shf[:], in0=vc[:, bi, m:m + 1], in1=shf[:],
                                            op=AL.subtract)
                    scls.append(scl); shfs.append(shf)
                return scls, shfs

            def mlp_two_layers(w1_idx, b1_row, w2_idx, b2_row, src_is_z0):
                """z2 = relu(l2(relu(l1(z)))) chunk-wise; z read/written via zdr.
                src_is_z0: first conv reads z0row (din=1) instead of zdr."""
                for q in range(8):
                    n0 = q * 512
                    z1q = mid.tile([128, 4, 512], F32, tag="z1q")
                    if src_is_z0:
                        for m in range(4):
                            ps = pp.tile([128, 512], F32, tag="psA")
                            nc.tensor.matmul(out=ps[:], lhsT=w1row[:, m * 128:(m + 1) * 128],
                                             rhs=z0row[:, n0:n0 + 512],
                                             start=True, stop=True)
                            nc.scalar.activation(out=z1q[:, m, :], in_=ps[:], func=AF.Relu,
                                                 bias=vc[:, b1_row, m:m + 1], scale=1.0)
                    else:
                        zq = mid.tile([128, 4, 512], F32, tag="zio")
                        nc.sync.dma_start(out=zq[:], in_=zdr[:, :, n0:n0 + 512])
                        for m2 in range(4):
                            psq = ppF.tile([128, 512], F32, tag="psF")
                            for k in range(4):
                                lw = spA.tile([128, 128], F32, tag="lw")
                                nc.sync.dma_start(
                                    out=lw[:],
                                    in_=t_Wm[w1_idx, k * 128:(k + 1) * 128,
                                             m2 * 128:(m2 + 1) * 128])
                                nc.tensor.matmul(out=psq[:], lhsT=lw[:],
                                                 rhs=zq[:, k, :],
                                                 start=(k == 0), stop=(k == 3))
                            nc.scalar.activation(out=z1q[:, m2, :], in_=psq[:], func=AF.Relu,
                                                 bias=vc[:, b1_row, m2:m2 + 1], scale=1.0)
                    z2q = mid.tile([128, 4, 512], F32, tag="z2q")
                    for m2 in range(4):
                        psq = ppF.tile([128, 512], F32, tag="psF")
                        for k in range(4):
                            lw = spA.tile([128, 128], F32, tag="lw")
                            nc.sync.dma_start(
                                out=lw[:],
                                in_=t_Wm[w2_idx, k * 128:(k + 1) * 128,
                                         m2 * 128:(m2 + 1) * 128])
                            nc.tensor.matmul(out=psq[:], lhsT=lw[:],
                                             rhs=z1q[:, k, :],
                                             start=(k == 0), stop=(k == 3))
                        nc.scalar.activation(out=z2q[:, m2, :], in_=psq[:], func=AF.Relu,
                                             bias=vc[:, b2_row, m2:m2 + 1], scale=1.0)
                    nc.sync.dma_start(out=zdr[:, :, n0:n0 + 512], in_=z2q[:])

            def zdr_chunk(q):
                zc = mid.tile([128, 4, 512], F32, tag="zio")
                nc.sync.dma_start(out=zc[:], in_=zdr[:, :, q * 512:(q + 1) * 512])
                return zc

            def hfm_chunk(q):
                return h_fm[:, :, q * 512:(q + 1) * 512]

            # ---------------- conv1 ----------------
            x_sb = sp.tile([128, GC], F32, tag="xsb")
            nc.sync.dma_start(out=x_sb[:], in_=t_x[:].rearrange("(g i) -> i g", i=128))
            agg0 = pp.tile([128, GC], F32, tag="psA")
            for g in range(GC):
                Ag = spA.tile([128, 128], F32, tag="Ast")
                nc.sync.dma_start(out=Ag[:], in_=Adr[g])
                nc.tensor.matmul(out=agg0[:, g:g + 1], lhsT=Ag[:], rhs=x_sb[:, g:g + 1],
                                 start=True, stop=True)
            z0 = sp.tile([128, GC], F32, tag="z0")
            nc.vector.tensor_scalar(out=z0[:], in0=x_sb[:], scalar1=float(e1),
                                    scalar2=None, op0=AL.mult)
            nc.vector.tensor_tensor(out=z0[:], in0=z0[:], in1=agg0[:], op=AL.add)
            nc.sync.dma_start(out=z0dr[:].rearrange("(g i) -> i g", i=128), in_=z0[:])
            z0row = rows.tile([1, NODES_C], F32, tag="rowA")
            nc.sync.dma_start(out=z0row[:], in_=z0dr[:].rearrange("(o n) -> o n", o=1))
            w1row = sp.tile([1, D], F32, tag="w1row")
            nc.sync.dma_start(out=w1row[:], in_=t_vecs[V_W1].rearrange("(o n) -> o n", o=1))

            mlp_two_layers(None, V_B1, 0, V_B2, src_is_z0=True)
            scls, shfs = bn_stats_allreduce(zdr_chunk, BN_EPS, "gin1")
            for q in range(8):
                zc = zdr_chunk(q)
                for m in range(4):
                    nc.vector.tensor_scalar(out=zc[:, m, :], in0=zc[:, m, :],
                                            scalar1=scls[m][:], scalar2=shfs[m][:],
                                            op0=AL.mult, op1=AL.add)
                    nc.scalar.activation(out=h_fm[:, m, q * 512:(q + 1) * 512],
                                         in_=zc[:, m, :], func=AF.Lrelu, alpha=LRELU_SLOPE)
            scls, shfs = bn_stats_allreduce(hfm_chunk, BN_EPS * NP, "bn1")
            for m in range(4):
                nc.vector.tensor_scalar(out=h_fm[:, m, :], in0=h_fm[:, m, :],
                                        scalar1=scls[m][:], scalar2=shfs[m][:],
                                        op0=AL.mult, op1=AL.add)

            # ---------------- conv loops ----------------
            for (wm1, wm2, b1r, b2r, ginkey, bnskey, ee) in (
                    (1, 2, V_C0B1, V_C0B2, "gin2", "bns0", e2),
                    (3, 4, V_C1B1, V_C1B2, "gin3", "bns1", e3)):
                # z = e*h + A@h (via per-graph transposes), written to zdr
                for g in range(GC):
                    tp = pp.tile([128, 4, 128], F32, tag="psA")
                    for m in range(4):
                        nc.tensor.transpose(out=tp[:, m, :],
                                            in_=h_fm[:, m, g * 128:(g + 1) * 128],
                                            identity=ident[:])
                    hng = sp.tile([128, 512], F32, tag="hng")
                    nc.vector.tensor_copy(out=hng[:], in_=tp[:].rearrange("p a b -> p (a b)"))
                    Ag = spA.tile([128, 128], F32, tag="Ast")
                    nc.sync.dma_start(out=Ag[:], in_=Adr[g])
                    aps = ppB.tile([128, 512], F32, tag="psC")
                    nc.tensor.matmul(out=aps[:], lhsT=Ag[:], rhs=hng[:],
                                     start=True, stop=True)
                    asb = sp.tile([128, 512], F32, tag="agg_sb")
                    nc.vector.tensor_copy(out=asb[:], in_=aps[:])
                    tp2 = ppB.tile([128, 4, 128], F32, tag="psC")
                    zg = sp.tile([128, 4, 128], F32, tag="zg")
                    for m in range(4):
                        nc.tensor.transpose(out=tp2[:, m, :],
                                            in_=asb[:, m * 128:(m + 1) * 128],
                                            identity=ident[:])
                        nc.vector.tensor_scalar(
                            out=zg[:, m, :],
                            in0=h_fm[:, m, g * 128:(g + 1) * 128],
                            scalar1=float(ee), scalar2=None, op0=AL.mult)
                        nc.vector.tensor_tensor(out=zg[:, m, :], in0=zg[:, m, :],
                                                in1=tp2[:, m, :], op=AL.add)
                    nc.sync.dma_start(out=zdr[:, :, g * 128:(g + 1) * 128], in_=zg[:])
                mlp_two_layers(wm1, b1r, wm2, b2r, src_is_z0=False)
                scls, shfs = bn_stats_allreduce(zdr_chunk, BN_EPS, ginkey)
                for q in range(8):
                    zc = zdr_chunk(q)
                    for m in range(4):
                        nc.vector.tensor_scalar(out=zc[:, m, :], in0=zc[:, m, :],
                                                scalar1=scls[m][:], scalar2=shfs[m][:],
                                                op0=AL.mult, op1=AL.add)
                        nc.scalar.activation(out=zc[:, m, :], in_=zc[:, m, :],
                                             func=AF.Lrelu, alpha=LRELU_SLOPE)
                        nc.vector.tensor_tensor(out=h_fm[:, m, q * 512:(q + 1) * 512],
                                                in0=h_fm[:, m, q * 512:(q + 1) * 512],
                                                in1=zc[:, m, :], op=AL.add)
                scls, shfs = bn_stats_allreduce(hfm_chunk, BN_EPS * NP, bnskey)
                for m in range(4):
                    nc.vector.tensor_scalar(out=h_fm[:, m, :], in0=h_fm[:, m, :],
                                            scalar1=scls[m][:], scalar2=shfs[m][:],
                                            op0=AL.mult, op1=AL.add)

            # ---------------- readout ----------------
            h64 = big.tile([64, NODES_C], F32, tag="scr")
            l1w_sb = consts.tile([128, 4, 64], F32)
            nc.sync.dma_start(out=l1w_sb[:], in_=t_l1w[:].rearrange("(k p) o -> p k o", p=128))
            for q in range(8):
                ps64 = pp.tile([64, 512], F32, tag="psA")
                for k in range(4):
                    nc.tensor.matmul(out=ps64[:], lhsT=l1w_sb[:, k, :],
                                     rhs=h_fm[:, k, q * 512:(q + 1) * 512],
                                     start=(k == 0), stop=(k == 3))
                nc.scalar.activation(out=h64[:, q * 512:(q + 1) * 512], in_=ps64[:],
                                     func=AF.Lrelu, alpha=LRELU_SLOPE,
                                     bias=vc[:64, V_L1B, 0:1], scale=1.0)
            l2w_sb = consts.tile([64, 1], F32)
            nc.sync.dma_start(out=l2w_sb[:], in_=t_vecs[V_L2W, 0:64, None])
            hlast = rows.tile([1, NODES_C], F32, tag="rowB")
            b2t = consts.tile([1, 1], F32)
            nc.vector.memset(b2t[:], float(lin2b))
            for q in range(8):
                psl = pp.tile([1, 512], F32, tag="psA")
                nc.tensor.matmul(out=psl[:], lhsT=l2w_sb[:], rhs=h64[:, q * 512:(q + 1) * 512],
                                 start=True, stop=True)
                nc.scalar.activation(out=hlast[:, q * 512:(q + 1) * 512], in_=psl[:],
                                     func=AF.Lrelu, alpha=LRELU_SLOPE, bias=b2t[:], scale=1.0)

            # minmax -> p
            hg = hlast[:].rearrange("o (g i) -> o g i", g=GC)
            bmax = sp.tile([1, GC], F32, tag="bmax")
            bmin = sp.tile([1, GC], F32, tag="bmin")
            nc.vector.tensor_reduce(bmax[:], hg, AX.X, AL.max)
            nc.vector.tensor_reduce(bmin[:], hg, AX.X, AL.min)
            denom = sp.tile([1, GC], F32, tag="denom")
            nc.vector.tensor_scalar(out=denom[:], in0=bmax[:], scalar1=1e-6, scalar2=None,
                                    op0=AL.add)
            nc.vector.tensor_tensor(out=denom[:], in0=denom[:], in1=bmin[:], op=AL.subtract)
            nc.vector.reciprocal(out=denom[:], in_=denom[:])
            prow = rows.tile([1, NODES_C], F32, tag="rowA")
            pg = prow[:].rearrange("o (g i) -> o g i", g=GC)
            for g in range(GC):
                nc.vector.tensor_scalar(out=pg[:, g, :], in0=hg[:, g, :],
                                        scalar1=bmin[:, g:g + 1], scalar2=denom[:, g:g + 1],
                                        op0=AL.subtract, op1=AL.mult)
            nc.sync.dma_start(out=o_p[:].rearrange("(o n) -> o n", o=1), in_=prow[:])
            nc.sync.dma_start(out=hladr[:].rearrange("(o n) -> o n", o=1), in_=prow[:])
            p_col = sp.tile([128, GC], F32, tag="pcol")
            nc.sync.dma_start(out=p_col[:], in_=hladr[:].rearrange("(g i) -> i g", i=128))

            # ---------------- per-graph quantities + laplacians ----------------
            dW = sp.tile([128, GC], F32, tag="dW")
            Wp_ps = pp.tile([128, GC], F32, tag="psA")
            deg = sp.tile([128, GC], F32, tag="deg")
            for g in range(GC):
                Wg = spA.tile([128, 128], F32, tag="Wst")
                nc.sync.dma_start(out=Wg[:], in_=Wdr[g])
                nc.vector.tensor_reduce(dW[:, g:g + 1], Wg[:], AX.X, AL.add)
                nc.tensor.matmul(out=Wp_ps[:, g:g + 1], lhsT=Wg[:], rhs=p_col[:, g:g + 1],
                                 start=True, stop=True)
            q_t = sp.tile([128, GC], F32, tag="qt")
            nc.vector.tensor_tensor(out=q_t[:], in0=dW[:], in1=p_col[:], op=AL.mult)
            nc.vector.tensor_tensor(out=q_t[:], in0=q_t[:], in1=Wp_ps[:], op=AL.subtract)
            nc.vector.tensor_tensor(out=q_t[:], in0=q_t[:], in1=p_col[:], op=AL.mult)
            pair_ps = pp.tile([1, GC], F32, tag="psA")
            nc.tensor.matmul(out=pair_ps[:], lhsT=ones_c[:], rhs=q_t[:], start=True, stop=True)
            gam_ps = pp.tile([1, GC], F32, tag="psB")
            nc.tensor.matmul(out=gam_ps[:], lhsT=ones_c[:], rhs=dW[:], start=True, stop=True)
            gamma = sp.tile([1, GC], F32, tag="gamma")
            nc.vector.tensor_scalar(out=gamma[:], in0=gam_ps[:], scalar1=0.5, scalar2=None,
                                    op0=AL.mult)
            frel = sp.tile([1, GC], F32, tag="frel")
            nc.vector.tensor_scalar(out=frel[:], in0=pair_ps[:], scalar1=-1.0,
                                    scalar2=None, op0=AL.mult)
            nc.vector.tensor_tensor(out=frel[:], in0=gamma[:], in1=frel[:], op=AL.add)
            nc.sync.dma_start(out=o_fr[:].rearrange("(o n) -> o n", o=1), in_=frel[:])

            omp = sp.tile([128, GC], F32, tag="omp")
            nc.vector.tensor_scalar(out=omp[:], in0=p_col[:], scalar1=-1.0, scalar2=1.0,
                                    op0=AL.mult, op1=AL.add)
            for g in range(GC):
                Ag = spA.tile([128, 128], F32, tag="Ast")
                nc.sync.dma_start(out=Ag[:], in_=Adr[g])
                nc.vector.tensor_reduce(deg[:, g:g + 1], Ag[:], AX.X, AL.add)
                l0 = sp.tile([128, 128], F32, tag="l0")
                nc.vector.tensor_scalar(out=l0[:], in0=ident[:], scalar1=deg[:, g:g + 1],
                                        scalar2=None, op0=AL.mult)
                nc.vector.tensor_tensor(out=l0[:], in0=l0[:], in1=Ag[:], op=AL.subtract)
                nc.sync.dma_start(out=lapdr[g].rearrange("(i j) -> i j", i=NP),
                                  in_=l0[:])
                pb_ps = ppB.tile([128, 128], F32, tag="psC")
                nc.tensor.matmul(out=pb_ps[:], lhsT=ones_r[:], rhs=pg[:, g, :],
                                 start=True, stop=True)
                bm = sp.tile([128, 128], F32, tag="bm")
                nc.vector.tensor_scalar(out=bm[:], in0=pb_ps[:], scalar1=omp[:, g:g + 1],
                                        scalar2=None, op0=AL.subtract)
                nc.scalar.activation(out=bm[:], in_=bm[:], func=AF.Square)
                nc.vector.tensor_tensor(out=bm[:], in0=bm[:], in1=Ag[:], op=AL.mult)
                degn = sp.tile([128, 1], F32, tag="degn")
                nc.vector.tensor_reduce(degn[:], bm[:], AX.X, AL.add)
                ln_ = sp.tile([128, 128], F32, tag="ln_")
                nc.vector.tensor_scalar(out=ln_[:], in0=ident[:], scalar1=degn[:],
                                        scalar2=None, op0=AL.mult)
                nc.vector.tensor_tensor(out=ln_[:], in0=ln_[:], in1=bm[:], op=AL.subtract)
                nc.sync.dma_start(out=lapdr[GC + g].rearrange("(i j) -> i j", i=NP),
                                  in_=ln_[:])

            # ---------------- Householder tridiagonalization ----------------
            M = 2 * GC
            L = big.tile([M, NP, NP], F32, tag="h_fm")
            nc.sync.dma_start(out=L[:], in_=lapdr[:].rearrange("m (i j) -> m i j", i=NP))
            tmp = big.tile([M, NP, 64], F32, tag="scr")
            alphas = consts.tile([M, NP], F32)
            betasq = consts.tile([M, NP], F32)
            nc.vector.memset(betasq[:], 1e-30)
            tiny1 = consts.tile([M, 1], F32)
            nc.vector.memset(tiny1[:], 1e-30)
            onesM = consts.tile([M, 1], F32)
            nc.vector.memset(onesM[:], 1.0)

            for k in range(HH_STEPS):
                r = NP - k - 1
                nc.vector.tensor_copy(out=alphas[:, k:k + 1], in_=L[:, k, k:k + 1])
                xk = sp.tile([M, NP], F32, tag="xk")
                nc.vector.tensor_copy(out=xk[:, :r], in_=L[:, k + 1:, k])
                sq = sp.tile([M, NP], F32, tag="sqk")
                nc.vector.tensor_tensor(out=sq[:, :r], in0=xk[:, :r], in1=xk[:, :r], op=AL.mult)
                nsq = sp.tile([M, 1], F32, tag="nsq")
                nc.vector.tensor_reduce(nsq[:], sq[:, :r], AX.X, AL.add)
                nc.vector.tensor_scalar(out=betasq[:, k:k + 1], in0=nsq[:], scalar1=tiny1[:],
                                        scalar2=None, op0=AL.max)
                nrm = sp.tile([M, 1], F32, tag="nrm")
                nc.scalar.activation(out=nrm[:], in_=nsq[:], func=AF.Sqrt)
                sg = sp.tile([M, 1], F32, tag="sg")
                nc.scalar.activation(out=sg[:], in_=xk[:, 0:1], func=AF.Sign)
                sgz = sp.tile([M, 1], U32, tag="sgz")
                nc.vector.tensor_scalar(out=sgz[:], in0=sg[:], scalar1=0.0, scalar2=None,
                                        op0=AL.is_equal)
                nc.vector.copy_predicated(sg[:], sgz[:], onesM[:])
                snr = sp.tile([M, 1], F32, tag="snr")
                nc.vector.tensor_tensor(out=snr[:], in0=sg[:], in1=nrm[:], op=AL.mult)
                nc.vector.tensor_tensor(out=xk[:, 0:1], in0=xk[:, 0:1], in1=snr[:], op=AL.add)
                nc.vector.tensor_tensor(out=sq[:, :r], in0=xk[:, :r], in1=xk[:, :r], op=AL.mult)
                vns = sp.tile([M, 1], F32, tag="vns")
                nc.vector.tensor_reduce(vns[:], sq[:, :r], AX.X, AL.add)
                nc.vector.tensor_scalar(out=vns[:], in0=vns[:], scalar1=tiny1[:],
                                        scalar2=None, op0=AL.max)
                tau = sp.tile([M, 1], F32, tag="tau")
                nc.vector.reciprocal(out=tau[:], in_=vns[:])
                nc.vector.tensor_scalar(out=tau[:], in0=tau[:], scalar1=2.0, scalar2=None,
                                        op0=AL.mult)
                ztau = sp.tile([M, 1], U32, tag="ztau")
                nc.vector.tensor_scalar(out=ztau[:], in0=nsq[:], scalar1=1e-20, scalar2=None,
                                        op0=AL.is_lt)
                zz = sp.tile([M, 1], F32, tag="zz")
                nc.vector.memset(zz[:], 0.0)
                nc.vector.copy_predicated(tau[:], ztau[:], zz[:])
                Lsub = L[:, k + 1:, k + 1:]
                wk = sp.tile([M, NP], F32, tag="wk")
                jsp = (r + 1) // 2
                for (j0, j1) in ((0, jsp), (jsp, r)):
                    jw = j1 - j0
                    if jw <= 0:
                        continue
                    nc.vector.tensor_tensor(
                        out=tmp[:, :r, :jw],
                        in0=L[:, k + 1:, k + 1 + j0:k + 1 + j1],
                        in1=xk[:, :r, None].broadcast_to([M, r, jw]),
                        op=AL.mult)
                    tTh = bass.AP(tensor=tmp.tensor, offset=tmp[:].offset,
                                  ap=[tmp[:].ap[0], [1, jw], [64, r]])
                    nc.vector.tensor_reduce(wk[:, j0:j1], tTh, AX.X, AL.add)
                nc.vector.tensor_tensor(out=sq[:, :r], in0=xk[:, :r], in1=wk[:, :r], op=AL.mult)
                Kd = sp.tile([M, 1], F32, tag="Kd")
                nc.vector.tensor_reduce(Kd[:], sq[:, :r], AX.X, AL.add)
                c2 = sp.tile([M, 1], F32, tag="c2")
                nc.vector.tensor_tensor(out=c2[:], in0=tau[:], in1=tau[:], op=AL.mult)
                nc.vector.tensor_tensor(out=c2[:], in0=c2[:], in1=Kd[:], op=AL.mult)
                nc.vector.tensor_scalar(out=c2[:], in0=c2[:], scalar1=0.5, scalar2=None,
                                        op0=AL.mult)
                uk = sp.tile([M, NP], F32, tag="uk")
                nc.vector.tensor_scalar(out=uk[:, :r], in0=wk[:, :r], scalar1=tau[:],
                                        scalar2=None, op0=AL.mult)
                nc.vector.tensor_scalar(out=sq[:, :r], in0=xk[:, :r], scalar1=c2[:],
                                        scalar2=None, op0=AL.mult)
                nc.vector.tensor_tensor(out=uk[:, :r], in0=uk[:, :r], in1=sq[:, :r],
                                        op=AL.subtract)
                for (j0, j1) in ((0, jsp), (jsp, r)):
                    jw = j1 - j0
                    if jw <= 0:
                        continue
                    Lh = L[:, k + 1:, k + 1 + j0:k + 1 + j1]
                    nc.vector.tensor_tensor(
                        out=tmp[:, :r, :jw],
                        in0=xk[:, :r, None].broadcast_to([M, r, jw]),
                        in1=uk[:, None, j0:j1].broadcast_to([M, r, jw]),
                        op=AL.mult)
                    nc.vector.tensor_tensor(out=Lh, in0=Lh, in1=tmp[:, :r, :jw],
                                            op=AL.subtract)
                    nc.vector.tensor_tensor(
                        out=tmp[:, :r, :jw],
                        in0=uk[:, :r, None].broadcast_to([M, r, jw]),
                        in1=xk[:, None, j0:j1].broadcast_to([M, r, jw]),
                        op=AL.mult)
                    nc.vector.tensor_tensor(out=Lh, in0=Lh, in1=tmp[:, :r, :jw],
                                            op=AL.subtract)

            nc.vector.tensor_copy(out=alphas[:, NP - 2:NP - 1], in_=L[:, NP - 2, NP - 2:NP - 1])
            nc.vector.tensor_copy(out=alphas[:, NP - 1:NP], in_=L[:, NP - 1, NP - 1:NP])
            lastb = sp.tile([M, 1], F32, tag="lastb")
            nc.vector.tensor_tensor(out=lastb[:], in0=L[:, NP - 1, NP - 2:NP - 1],
                                    in1=L[:, NP - 1, NP - 2:NP - 1], op=AL.mult)
            nc.vector.tensor_scalar(out=betasq[:, NP - 2:NP - 1], in0=lastb[:],
                                    scalar1=tiny1[:], scalar2=None, op0=AL.max)

            # ---------------- Sturm multisection ----------------
            nc.sync.dma_start(out=abdr[0], in_=alphas[:])
            nc.sync.dma_start(out=abdr[1], in_=betasq[:])
            al_b = consts.tile([NPROB, NP], F32)
            bs_b = consts.tile([NPROB, NP], F32)
            nc.sync.dma_start(out=al_b[:M, :], in_=abdr[0])
            nc.sync.dma_start(out=al_b[M:, :], in_=abdr[0, GC:, :])
            nc.sync.dma_start(out=bs_b[:M, :], in_=abdr[1])
            nc.sync.dma_start(out=bs_b[M:, :], in_=abdr[1, GC:, :])
            tgt = consts.tile([NPROB, 1], F32)
            nc.vector.memset(tgt[:GC], 3.0)
            nc.vector.memset(tgt[GC:M], 2.0)
            nc.vector.memset(tgt[M:], 3.0)
            amax = sp.tile([NPROB, 1], F32, tag="amax")
            nc.vector.tensor_reduce(amax[:], al_b[:], AX.X, AL.max)
            bmx = sp.tile([NPROB, 1], F32, tag="bmx")
            nc.vector.tensor_reduce(bmx[:], bs_b[:], AX.X, AL.max)
            nc.scalar.activation(out=bmx[:], in_=bmx[:], func=AF.Sqrt)
            hi = sp.tile([NPROB, 1], F32, tag="hi")
            nc.vector.tensor_scalar(out=hi[:], in0=bmx[:], scalar1=2.0, scalar2=None,
                                    op0=AL.mult)
            nc.vector.tensor_tensor(out=hi[:], in0=hi[:], in1=amax[:], op=AL.add)
            nc.vector.tensor_scalar(out=hi[:], in0=hi[:], scalar1=1.001, scalar2=0.01,
                                    op0=AL.mult, op1=AL.add)
            lo = sp.tile([NPROB, 1], F32, tag="lo")
            nc.vector.tensor_scalar(out=lo[:], in0=hi[:], scalar1=-0.01, scalar2=None,
                                    op0=AL.mult)
            iotaS = consts.tile([NPROB, BIS_S], F32)
            nc.gpsimd.iota(iotaS[:], pattern=[[1, BIS_S]], base=1, channel_multiplier=0,
                           allow_small_or_imprecise_dtypes=True)
            estore = big.tile([NPROB, NP, BIS_S], F32, tag="scr")
            negb = consts.tile([NPROB, NP], F32)
            nc.vector.tensor_scalar(out=negb[:], in0=bs_b[:], scalar1=-1.0, scalar2=None,
                                    op0=AL.mult)
            nega = consts.tile([NPROB, NP], F32)
            nc.vector.tensor_scalar(out=nega[:], in0=al_b[:], scalar1=-1.0, scalar2=None,
                                    op0=AL.mult)

            for _ in range(BIS_PASSES):
                stepw = sp.tile([NPROB, 1], F32, tag="stepw")
                nc.vector.tensor_tensor(out=stepw[:], in0=hi[:], in1=lo[:], op=AL.subtract)
                nc.vector.tensor_scalar(out=stepw[:], in0=stepw[:],
                                        scalar1=float(1.0 / (BIS_S + 1)), scalar2=None,
                                        op0=AL.mult)
                sig = sp.tile([NPROB, BIS_S], F32, tag="sig")
                nc.vector.tensor_scalar(out=sig[:], in0=iotaS[:], scalar1=stepw[:],
                                        scalar2=lo[:], op0=AL.mult, op1=AL.add)
                nc.vector.tensor_scalar(out=estore[:, 0, :], in0=sig[:],
                                        scalar1=al_b[:, 0:1], scalar2=None, op0=AL.subtract)
                for i in range(1, NP):
                    rec = sp.tile([NPROB, BIS_S], F32, tag="rec")
                    nc.vector.reciprocal(out=rec[:], in_=estore[:, i - 1, :])
                    nc.vector.tensor_scalar(out=estore[:, i, :], in0=rec[:],
                                            scalar1=negb[:, i - 1:i], scalar2=nega[:, i:i + 1],
                                            op0=AL.mult, op1=AL.add)
                    nc.vector.tensor_tensor(out=estore[:, i, :], in0=estore[:, i, :],
                                            in1=sig[:], op=AL.add)
                nc.vector.tensor_scalar(out=estore[:], in0=estore[:], scalar1=0.0,
                                        scalar2=None, op0=AL.is_gt)
                cT = bass.AP(tensor=estore.tensor, offset=estore[:].offset,
                             ap=[estore[:].ap[0], [1, BIS_S], [BIS_S, NP]])
                cnt = sp.tile([NPROB, BIS_S], F32, tag="cnt")
                nc.vector.tensor_reduce(cnt[:], cT, AX.X, AL.add)
                meets = sp.tile([NPROB, BIS_S], F32, tag="meets")
                nc.vector.tensor_scalar(out=meets[:], in0=cnt[:], scalar1=tgt[:],
                                        scalar2=None, op0=AL.is_ge)
                selt = sp.tile([NPROB, BIS_S], F32, tag="selt")
                nc.vector.tensor_scalar(out=selt[:], in0=meets[:], scalar1=-1e6,
                                        scalar2=1e6, op0=AL.mult, op1=AL.add)
                nc.vector.tensor_tensor(out=selt[:], in0=selt[:], in1=iotaS[:], op=AL.add)
                sstar = sp.tile([NPROB, 1], F32, tag="sstar")
                nc.vector.tensor_reduce(sstar[:], selt[:], AX.X, AL.min)
                smax = sp.tile([NPROB, 1], F32, tag="smax")
                nc.vector.memset(smax[:], float(BIS_S + 1))
                nc.vector.tensor_tensor(out=sstar[:], in0=sstar[:], in1=smax[:], op=AL.min)
                nc.vector.tensor_scalar(out=sstar[:], in0=sstar[:], scalar1=-1.0,
                                        scalar2=None, op0=AL.add)
                nc.vector.tensor_scalar(out=sstar[:], in0=sstar[:], scalar1=stepw[:],
                                        scalar2=None, op0=AL.mult)
                nc.vector.tensor_tensor(out=lo[:], in0=lo[:], in1=sstar[:], op=AL.add)
                nc.vector.tensor_tensor(out=hi[:], in0=lo[:], in1=stepw[:], op=AL.add)

            lam = sp.tile([NPROB, 1], F32, tag="lam")
            nc.vector.tensor_tensor(out=lam[:], in0=lo[:], in1=hi[:], op=AL.add)
            nc.vector.tensor_scalar(out=lam[:], in0=lam[:], scalar1=0.5, scalar2=None,
                                    op0=AL.mult)
            nc.sync.dma_start(out=lamdr[:, None], in_=lam[:])
            lrow = rows.tile([1, NPROB], F32, tag="rowC")
            nc.sync.dma_start(out=lrow[:], in_=lamdr[:].rearrange("(o n) -> o n", o=1))

            aaa = sp.tile([1, GC], F32, tag="aaa")
            nc.vector.reciprocal(out=aaa[:], in_=lrow[:, 0:GC])
            nc.vector.tensor_scalar(out=aaa[:], in0=aaa[:], scalar1=float(-np.log(1e-4)),
                                    scalar2=None, op0=AL.mult)
            d21 = sp.tile([1, GC], F32, tag="d21")
            nc.vector.tensor_tensor(out=d21[:], in0=lrow[:, M:], in1=lrow[:, GC:M],
                                    op=AL.subtract)
            nc.vector.tensor_tensor(out=d21[:], in0=d21[:], in1=aaa[:], op=AL.mult)
            nc.vector.tensor_scalar(out=d21[:], in0=d21[:], scalar1=-1.0, scalar2=None,
                                    op0=AL.mult)
            grel = sp.tile([1, GC], F32, tag="grel")
            nc.scalar.activation(out=grel[:], in_=d21[:], func=AF.Exp)
            nc.sync.dma_start(out=o_gr[:].rearrange("(o n) -> o n", o=1), in_=grel[:])
            lossr = sp.tile([1, GC], F32, tag="lossr")
            nc.vector.tensor_tensor(out=lossr[:], in0=gamma[:], in1=grel[:], op=AL.mult)
            fterm = sp.tile([1, GC], F32, tag="fterm")
            nc.vector.tensor_scalar(out=fterm[:], in0=frel[:], scalar1=float(120.0 / 36.0),
                                    scalar2=None, op0=AL.mult)
            nc.vector.tensor_tensor(out=lossr[:], in0=lossr[:], in1=fterm[:], op=AL.add)
            nc.sync.dma_start(out=o_loss[:].rearrange("(o n) -> o n", o=1), in_=lossr[:])

    _split_waits(nc)
    return nc


def kernel(x, edge_attr, params, edge_index, batch, edge_batch_index):
    import sys, os
    sys.path.insert(0, os.path.dirname(os.path.abspath(__file__)))
    from concourse.bass_utils import run_bass_kernel_spmd

    f32 = np.float32
    x = np.asarray(x, f32)
    ea = np.asarray(edge_attr, f32)
    ei = np.asarray(edge_index).astype(np.int64)
    batch_np = np.asarray(batch).astype(np.int64)

    counts = np.bincount(batch_np, minlength=G)
    uniform = bool((counts == NP).all())

    row = ei[0]
    E = G * E_HALF
    src, dst = row[:E], ei[1][:E]
    g_of = src // NP
    lr = (src - g_of * NP).astype(f32)
    lc = (dst - g_of * NP).astype(f32)
    w_half = ea[:E].astype(f32)

    def P(t):
        return np.asarray(t, f32)

    pr = params
    consts_in = (
        1.0 + float(np.asarray(pr["conv1"]["eps"])),
        1.0 + float(np.asarray(pr["convs"][0]["eps"])),
        1.0 + float(np.asarray(pr["convs"][1]["eps"])),
        float(np.asarray(pr["lin2"]["b"]).reshape(-1)[0]),
    )

    Wm = np.stack([
        P(pr["conv1"]["l2"]["w"]),
        P(pr["convs"][0]["l1"]["w"]), P(pr["convs"][0]["l2"]["w"]),
        P(pr["convs"][1]["l1"]["w"]), P(pr["convs"][1]["l2"]["w"]),
    ])
    vecs = np.zeros((21, D), f32)
    vecs[0] = P(pr["conv1"]["l1"]["w"]).reshape(-1)
    vecs[1] = P(pr["conv1"]["l1"]["b"]); vecs[2] = P(pr["conv1"]["l2"]["b"])
    vecs[3] = P(pr["convs"][0]["l1"]["b"]); vecs[4] = P(pr["convs"][0]["l2"]["b"])
    vecs[5] = P(pr["convs"][1]["l1"]["b"]); vecs[6] = P(pr["convs"][1]["l2"]["b"])
    vecs[7] = P(pr["conv1"]["bn_g"]); vecs[8] = P(pr["conv1"]["bn_b"])
    vecs[9] = P(pr["bn1_g"]); vecs[10] = P(pr["bn1_b"])
    vecs[11] = P(pr["convs"][0]["bn_g"]); vecs[12] = P(pr["convs"][0]["bn_b"])
    vecs[13] = P(pr["bns"][0]["g"]); vecs[14] = P(pr["bns"][0]["b"])
    vecs[15] = P(pr["convs"][1]["bn_g"]); vecs[16] = P(pr["convs"][1]["bn_b"])
    vecs[17] = P(pr["bns"][1]["g"]); vecs[18] = P(pr["bns"][1]["b"])
    vecs[19, :64] = P(pr["lin1"]["b"])
    vecs[20, :64] = P(pr["lin2"]["w"]).reshape(-1)

    key = ("prog", consts_in, uniform)
    if key not in _cache:
        _cache[key] = _build_program(consts_in, uniform)
    nc = _cache[key]

    in_maps = []
    for c in range(NCORES):
        gs = c * GC
        es = gs * E_HALF
        ns = gs * NP
        in_maps.append({
            "lr": lr[es:es + EC], "lc": lc[es:es + EC], "ew": w_half[es:es + EC],
            "x0": x[ns:ns + NODES_C],
            "Wm": Wm, "lin1w": P(pr["lin1"]["w"]), "vecs": vecs,
        })
    res = run_bass_kernel_spmd(nc, in_maps, core_ids=list(range(NCORES)))
    p_full = np.concatenate([res.results[c]["p_out"] for c in range(NCORES)])
    loss = np.concatenate([res.results[c]["loss"] for c in range(NCORES)])
    f_relax = np.concatenate([res.results[c]["f_relax"] for c in range(NCORES)])
    g_relax = np.concatenate([res.results[c]["g_relax"] for c in range(NCORES)])
    loss_mean = np.float32(loss.mean())
    return (p_full.astype(f32), loss.astype(f32), loss_mean,
            f_relax.astype(f32), g_relax.astype(f32))
